# revision 3
# baseline (speedup 1.0000x reference)
"""Single-head attention with QKV projections on 8 TRN2 NeuronCores.

Problem: B=4, S=2048, E=A=1024 f32.
  q = query @ Wq + bq ; k = key @ Wk + bk ; v = value @ Wv + bv
  out = softmax(q k^T / sqrt(A)) v

Sharding: pure data-parallel over (batch, query-half) -> 8 shards, no
collectives. Each core computes K/V projections for its batch (duplicated
across the 2 cores sharing a batch) and attention for 1024 queries.

Device-side layout strategy (per core):
  - Host pre-transposes activations so every matmul contracts over the
    partition axis with no on-chip transposes:
      xq = query_shard^T [E, 1024], xk = key_b^T [E, 2048], xv = value_b^T.
  - Projections produce qT [A, Sq] and kT [A, Sk] (A on partitions) and
    v [Sk, A] (natural).
  - Scores are computed TRANSPOSED: sT[k, q] = kT_tile^T @ qT, so that
    E = exp(sT/sqrt(A)) is directly the lhsT of the probs@V matmul - no
    transpose of the probability matrix and no partition-axis softmax
    reductions. The row-max subtraction is skipped (|scores| <= ~6 for
    this distribution, exp is safe in f32) making the softmax a plain
    exp/sum. Denominators come from a ones-column matmul; the 1/denom
    scale is folded into the PSUM->SBUF copy of the output.
  - All matmul operands are float32r (TF32-like, full PE rate at N>=512,
    ~1.5e-4 matmul rel err vs 2.3e-3 for bf16).

kT is bounced through a DRAM scratch tensor (SBUF cannot hold qT + v + kT
+ E at once); everything else stays resident.
"""
import sys

sys.path.insert(0, "/opt/trn_rl_repo")

import numpy as np

import concourse.bass as bass
import concourse.tile as tile
from concourse import bacc, bass_utils, mybir

B, S, E, A = 4, 2048, 1024, 1024
SQ = 1024          # queries per core
ET, AT = 8, 8      # 128-tiles of E and A
ST, KT, KC = 16, 16, 4  # 128-tiles of Sk; k-chunks of 512
QC, QS, AC = 2, 8, 2    # q 512-chunks, q 128-subtiles, a 512-chunks
SCALE = 1.0 / 32.0      # 1/sqrt(A)

f32 = mybir.dt.float32
f32r = mybir.dt.float32r
ts = bass.ts


def build():
    nc = bacc.Bacc("TRN2", target_bir_lowering=False, debug=False)
    Act = mybir.ActivationFunctionType
    Alu = mybir.AluOpType

    xq_d = nc.dram_tensor("xq", [E, SQ], f32r, kind="ExternalInput")
    xk_d = nc.dram_tensor("xk", [E, S], f32r, kind="ExternalInput")
    xv_d = nc.dram_tensor("xv", [E, S], f32r, kind="ExternalInput")
    wq_d = nc.dram_tensor("wq", [E, A], f32r, kind="ExternalInput")
    wk_d = nc.dram_tensor("wk", [E, A], f32r, kind="ExternalInput")
    wv_d = nc.dram_tensor("wv", [E, A], f32r, kind="ExternalInput")
    bqt_d = nc.dram_tensor("bqt", [128, AT], f32, kind="ExternalInput")
    bkt_d = nc.dram_tensor("bkt", [128, AT], f32, kind="ExternalInput")
    bvb_d = nc.dram_tensor("bvb", [128, A], f32, kind="ExternalInput")
    ones_d = nc.dram_tensor("ones", [128, 2], f32r, kind="ExternalInput")
    out_d = nc.dram_tensor("out", [SQ, A], f32, kind="ExternalOutput")

    with tile.TileContext(nc) as tc:
        with (
            tc.tile_pool(name="pers", bufs=1) as pers,
            tc.tile_pool(name="pqt", bufs=1) as pqt,
            tc.tile_pool(name="pv", bufs=1) as pv,
            tc.tile_pool(name="pstag", bufs=2) as pstag,
            tc.tile_pool(name="pp512", bufs=3, space="PSUM") as pp512,
            tc.tile_pool(name="pps", bufs=2, space="PSUM") as pps,
            tc.tile_pool(name="ppd", bufs=1, space="PSUM") as ppd,
            tc.tile_pool(name="pdram", bufs=1, space="DRAM") as pdram,
        ):
            bqt = pers.tile([128, AT], f32)
            nc.sync.dma_start(bqt[:], bqt_d.ap()[:, :])
            bkt = pers.tile([128, AT], f32)
            nc.sync.dma_start(bkt[:], bkt_d.ap()[:, :])
            bvb = pers.tile([128, A], f32)
            nc.sync.dma_start(bvb[:], bvb_d.ap()[:, :])
            ones = pers.tile([128, 2], f32r)
            nc.sync.dma_start(ones[:], ones_d.ap()[:, :])
            recip = pers.tile([128, QS], f32)
            acc = pers.tile([128, SQ], f32r)

            qT = pqt.tile([128, AT, SQ], f32r)      # [a-part, a-tile, q]
            v_sb = pv.tile([128, ST, A], f32r)      # [k-part, k-tile, a]
            ktp = pdram.tile([A, S], f32r)          # kT scratch [a, k]

            with tc.tile_pool(name="pw", bufs=1) as pw:
                # ---- Phase A: qT[a, q] = (query @ Wq + bq)^T ----
                with tc.tile_pool(name="pxq", bufs=1) as pxq:
                    wq = pw.tile([128, ET, A], f32r, tag="w", name="wq_t")
                    for et in range(ET):
                        nc.sync.dma_start(wq[:, et, :], wq_d.ap()[ts(et, 128), :])
                    xq_t = pxq.tile([128, ET, SQ], f32r)
                    for et in range(ET):
                        nc.sync.dma_start(xq_t[:, et, :], xq_d.ap()[ts(et, 128), :])
                    for at in range(AT):
                        for qc in range(QC):
                            ps = pp512.tile([128, 512], f32, tag="ps", name="ps_a")
                            for et in range(ET):
                                nc.tensor.matmul(
                                    ps[:], wq[:, et, ts(at, 128)],
                                    xq_t[:, et, ts(qc, 512)],
                                    start=(et == 0), stop=(et == ET - 1),
                                )
                            nc.scalar.activation(
                                qT[:, at, ts(qc, 512)], ps[:], Act.Identity,
                                bias=bqt[:, at:at + 1], scale=1.0,
                            )

                # ---- Phase B: v[s, a] = value @ Wv (bias added at the end) ----
                with tc.tile_pool(name="pxv", bufs=3) as pxv:
                    wv = pw.tile([128, ET, A], f32r, tag="w", name="wv_t")
                    for et in range(ET):
                        nc.sync.dma_start(wv[:, et, :], wv_d.ap()[ts(et, 128), :])
                    for st in range(ST):
                        xv_t = pxv.tile([128, ET, 128], f32r, tag="xv", name="xv_t")
                        for et in range(ET):
                            nc.sync.dma_start(
                                xv_t[:, et, :], xv_d.ap()[ts(et, 128), ts(st, 128)])
                        for ac in range(AC):
                            ps = pp512.tile([128, 512], f32, tag="ps", name="ps_b")
                            for et in range(ET):
                                nc.tensor.matmul(
                                    ps[:], xv_t[:, et, :], wv[:, et, ts(ac, 512)],
                                    start=(et == 0), stop=(et == ET - 1),
                                )
                            nc.any.tensor_copy(v_sb[:, st, ts(ac, 512)], ps[:])

                # ---- Phase B': kT[a, k] = (key @ Wk + bk)^T -> DRAM scratch ----
                with tc.tile_pool(name="pxk", bufs=2) as pxk:
                    wk = pw.tile([128, ET, A], f32r, tag="w", name="wk_t")
                    for et in range(ET):
                        nc.sync.dma_start(wk[:, et, :], wk_d.ap()[ts(et, 128), :])
                    for kc in range(KC):
                        xk_t = pxk.tile([128, ET, 512], f32r, tag="xk", name="xk_t")
                        for et in range(ET):
                            nc.sync.dma_start(
                                xk_t[:, et, :], xk_d.ap()[ts(et, 128), ts(kc, 512)])
                        for at in range(AT):
                            ps = pp512.tile([128, 512], f32, tag="ps", name="ps_k")
                            for et in range(ET):
                                nc.tensor.matmul(
                                    ps[:], wk[:, et, ts(at, 128)], xk_t[:, et, :],
                                    start=(et == 0), stop=(et == ET - 1),
                                )
                            kst = pstag.tile([128, 512], f32r, tag="kst", name="kst")
                            nc.scalar.activation(
                                kst[:], ps[:], Act.Identity,
                                bias=bkt[:, at:at + 1], scale=1.0,
                            )
                            nc.sync.dma_start(ktp[ts(at, 128), ts(kc, 512)], kst[:])

            # ---- Phase C: scores^T, exp, denominators, probs @ V ----
            with (
                tc.tile_pool(name="pe", bufs=1) as pe,
                tc.tile_pool(name="pktp", bufs=2) as pktp,
            ):
                E_t = pe.tile([128, KT, SQ], f32r)   # exp(scores^T) [k-part, kt, q]
                for kc in range(KC * 2):
                    ktp_t = pktp.tile([128, AT, 256], f32r, tag="ktp", name="ktp_t")
                    for at in range(AT):
                        nc.sync.dma_start(
                            ktp_t[:, at, :], ktp[ts(at, 128), ts(kc, 256)])
                    for ki in range(2):
                        kt = kc * 2 + ki
                        psc = pps.tile([128, SQ], f32, tag="psc", name="psc")
                        for at in range(AT):
                            for qc in range(QC):
                                nc.tensor.matmul(
                                    psc[:, ts(qc, 512)],
                                    ktp_t[:, at, ts(ki, 128)],
                                    qT[:, at, ts(qc, 512)],
                                    start=(at == 0), stop=(at == AT - 1),
                                )
                        nc.scalar.activation(
                            E_t[:, kt, :], psc[:], Act.Exp, bias=0.0, scale=SCALE)

                # denominators: acc[p, q] = sum_kt E[p, kt, q]; then
                # denom[qs] = acc[:, qs]^T @ 1 via PE; recip = 1/denom
                nc.vector.tensor_tensor(
                    acc[:], E_t[:, 0, :], E_t[:, 1, :], mybir.AluOpType.add)
                for kt in range(2, KT):
                    nc.vector.tensor_tensor(
                        acc[:], acc[:], E_t[:, kt, :], mybir.AluOpType.add)
                for qs in range(QS):
                    psd = ppd.tile([128, 2], f32, tag="psd", name="psd")
                    nc.tensor.matmul(
                        psd[:], acc[:, ts(qs, 128)], ones[:], start=True, stop=True)
                    nc.vector.reciprocal(recip[:, qs:qs + 1], psd[:, 0:1])

                for ac in range(AC):
                    for qs in range(QS):
                        ps = pp512.tile([128, 512], f32, tag="ps", name="ps_av")
                        for kt in range(KT):
                            nc.tensor.matmul(
                                ps[:], E_t[:, kt, ts(qs, 128)],
                                v_sb[:, kt, ts(ac, 512)],
                                start=(kt == 0), stop=(kt == KT - 1),
                            )
                        ot = pstag.tile([128, 512], f32, tag="ot", name="ot")
                        nc.vector.tensor_scalar(
                            ot[:], ps[:], recip[:, qs:qs + 1], None,
                            mybir.AluOpType.mult)
                        nc.vector.tensor_tensor(
                            ot[:], ot[:], bvb[:, ts(ac, 512)], mybir.AluOpType.add)
                        nc.sync.dma_start(out_d.ap()[ts(qs, 128), ts(ac, 512)], ot[:])

    nc.compile()
    return nc


_nc_cache = None


def _get_nc():
    global _nc_cache
    if _nc_cache is None:
        _nc_cache = build()
    return _nc_cache


def kernel(query, key, value, Wq, bq, Wk, bk, Wv, bv):
    query = np.asarray(query, dtype=np.float32)
    key = np.asarray(key, dtype=np.float32)
    value = np.asarray(value, dtype=np.float32)
    Wq = np.ascontiguousarray(np.asarray(Wq, dtype=np.float32))
    Wk = np.ascontiguousarray(np.asarray(Wk, dtype=np.float32))
    Wv = np.ascontiguousarray(np.asarray(Wv, dtype=np.float32))
    bq = np.asarray(bq, dtype=np.float32)
    bk = np.asarray(bk, dtype=np.float32)
    bv = np.asarray(bv, dtype=np.float32)

    nc = _get_nc()

    bqt = np.ascontiguousarray(bq.reshape(AT, 128).T)
    bkt = np.ascontiguousarray(bk.reshape(AT, 128).T)
    bvb = np.ascontiguousarray(np.broadcast_to(bv, (128, A)))
    ones = np.ones((128, 2), np.float32)

    kTs = [np.ascontiguousarray(key[b].T) for b in range(B)]
    vTs = [np.ascontiguousarray(value[b].T) for b in range(B)]

    in_maps = []
    for c in range(8):
        b, h = c // 2, c % 2
        in_maps.append({
            "xq": np.ascontiguousarray(query[b, h * SQ:(h + 1) * SQ, :].T),
            "xk": kTs[b],
            "xv": vTs[b],
            "wq": Wq, "wk": Wk, "wv": Wv,
            "bqt": bqt, "bkt": bkt, "bvb": bvb, "ones": ones,
        })

    global _last_in_maps
    _last_in_maps = in_maps
    res = bass_utils.run_bass_kernel_spmd(nc, in_maps, core_ids=list(range(8)))

    out = np.empty((B, S, A), np.float32)
    for c in range(8):
        b, h = c // 2, c % 2
        out[b, h * SQ:(h + 1) * SQ, :] = res.results[c]["out"]
    return out


# revision 5
# speedup vs baseline: 1.0656x; 1.0656x over previous
"""Single-head attention with QKV projections on 8 TRN2 NeuronCores.

Problem: B=4, S=2048, E=A=1024 f32.
  q = query @ Wq + bq ; k = key @ Wk + bk ; v = value @ Wv + bv
  out = softmax(q k^T / sqrt(A)) v

Sharding: pure data-parallel over (batch, query-half) -> 8 shards, no
collectives. Each core computes K/V projections for its batch (duplicated
across the 2 cores sharing a batch) and attention for 1024 queries.

Device-side layout strategy (per core):
  - Host pre-transposes activations so every matmul contracts over the
    partition axis with no on-chip transposes:
      xq = query_shard^T [E, 1024], xk = key_b^T [E, 2048], xv = value_b^T.
  - Projections produce qT [A, Sq] and kT [A, Sk] (A on partitions) and
    v [Sk, A] (natural).
  - Scores are computed TRANSPOSED: sT[k, q] = kT_tile^T @ qT, so that
    E = exp(sT/sqrt(A)) is directly the lhsT of the probs@V matmul - no
    transpose of the probability matrix and no partition-axis softmax
    reductions. The row-max subtraction is skipped (|scores| <= ~6 for
    this distribution, exp is safe in f32) making the softmax a plain
    exp/sum. Denominators come from a ones-column matmul on a
    DVE-prereduced tile; the 1/denom scale is folded into the
    PSUM->SBUF copy of the output.
  - All matmul operands are float32r (TF32-like, full PE rate at N>=512,
    ~1.5e-4 matmul rel err vs 2.3e-3 for bf16).

kT is bounced through a DRAM scratch tensor (SBUF cannot hold qT + v + kT
+ E at once); everything else stays resident. Wq/Wv/Wk live in separate
pools whose loads are issued a phase early so no phase boundary stalls on
a weight DMA. All streaming transfers use >=2KB per-partition rows (DMA
packet efficiency).
"""
import sys

sys.path.insert(0, "/opt/trn_rl_repo")

import numpy as np

import concourse.bass as bass
import concourse.tile as tile
from concourse import bacc, bass_utils, mybir

B, S, E, A = 4, 2048, 1024, 1024
SQ = 1024          # queries per core
ET, AT = 8, 8      # 128-tiles of E and A
ST, KT, KC = 16, 16, 4  # 128-tiles of Sk; k-chunks of 512
QC, QS, AC = 2, 8, 2    # q 512-chunks, q 128-subtiles, a 512-chunks
SCALE = 1.0 / 32.0      # 1/sqrt(A)

f32 = mybir.dt.float32
f32r = mybir.dt.float32r
ts = bass.ts


def build():
    nc = bacc.Bacc("TRN2", target_bir_lowering=False, debug=False)
    Act = mybir.ActivationFunctionType
    Alu = mybir.AluOpType

    xq_d = nc.dram_tensor("xq", [E, SQ], f32r, kind="ExternalInput")
    xk_d = nc.dram_tensor("xk", [E, S], f32r, kind="ExternalInput")
    xv_d = nc.dram_tensor("xv", [E, S], f32r, kind="ExternalInput")
    wq_d = nc.dram_tensor("wq", [E, A], f32r, kind="ExternalInput")
    wk_d = nc.dram_tensor("wk", [E, A], f32r, kind="ExternalInput")
    wv_d = nc.dram_tensor("wv", [E, A], f32r, kind="ExternalInput")
    bqt_d = nc.dram_tensor("bqt", [128, AT], f32, kind="ExternalInput")
    bkt_d = nc.dram_tensor("bkt", [128, AT], f32, kind="ExternalInput")
    bvb_d = nc.dram_tensor("bvb", [128, A], f32, kind="ExternalInput")
    ones_d = nc.dram_tensor("ones", [128, 2], f32r, kind="ExternalInput")
    out_d = nc.dram_tensor("out", [SQ, A], f32, kind="ExternalOutput")

    with tile.TileContext(nc) as tc:
        with (
            tc.tile_pool(name="pers", bufs=1) as pers,
            tc.tile_pool(name="pqt", bufs=1) as pqt,
            tc.tile_pool(name="pp512", bufs=3, space="PSUM") as pp512,
            tc.tile_pool(name="pps", bufs=2, space="PSUM") as pps,
            tc.tile_pool(name="ppd", bufs=1, space="PSUM") as ppd,
            tc.tile_pool(name="pdram", bufs=1, space="DRAM") as pdram,
        ):
            bqt = pers.tile([128, AT], f32)
            nc.gpsimd.dma_start(bqt[:], bqt_d.ap()[:, :])
            bkt = pers.tile([128, AT], f32)
            nc.gpsimd.dma_start(bkt[:], bkt_d.ap()[:, :])
            ones = pers.tile([128, 2], f32r)
            nc.gpsimd.dma_start(ones[:], ones_d.ap()[:, :])
            recip = pers.tile([128, QS], f32)

            qT = pqt.tile([128, AT, SQ], f32r)      # [a-part, a-tile, q]
            ktp = pdram.tile([A, S], f32r)          # kT scratch [a, k]

            # LIFO pool stack: pv and pwv open for (nearly) the whole kernel;
            # phase-local pools nest inside. wv/wk DMAs issue a phase early.
            pv = tc.alloc_tile_pool(name="pv", bufs=1)
            v_sb = pv.tile([128, ST, A], f32r)      # [k-part, k-tile, a]
            pwv = tc.alloc_tile_pool(name="pwv", bufs=1)
            wv = pwv.tile([128, ET, A], f32r)
            for et in range(ET):
                nc.scalar.dma_start(wv[:, et, :], wv_d.ap()[ts(et, 128), :])

            # ---- Phase A: qT[a, q] = (query @ Wq + bq)^T ----
            pwq = tc.alloc_tile_pool(name="pwq", bufs=1)
            wq = pwq.tile([128, ET, A], f32r)
            for et in range(ET):
                nc.scalar.dma_start(wq[:, et, :], wq_d.ap()[ts(et, 128), :])
            pxq = tc.alloc_tile_pool(name="pxq", bufs=1)
            xq_t = pxq.tile([128, ET, SQ], f32r)
            for et in range(ET):
                nc.sync.dma_start(xq_t[:, et, :], xq_d.ap()[ts(et, 128), :])
            for at in range(AT):
                for qc in range(QC):
                    ps = pp512.tile([128, 512], f32, tag="ps", name="ps_a")
                    for et in range(ET):
                        nc.tensor.matmul(
                            ps[:], wq[:, et, ts(at, 128)],
                            xq_t[:, et, ts(qc, 512)],
                            start=(et == 0), stop=(et == ET - 1),
                        )
                    nc.scalar.activation(
                        qT[:, at, ts(qc, 512)], ps[:], Act.Identity,
                        bias=bqt[:, at:at + 1], scale=1.0,
                    )
            pxq.release()
            pwq.release()

            # ---- Phase B: v[s, a] = value @ Wv (bias added at the end) ----
            pwk = tc.alloc_tile_pool(name="pwk", bufs=1)
            wk = pwk.tile([128, ET, A], f32r)
            for et in range(ET):
                nc.scalar.dma_start(wk[:, et, :], wk_d.ap()[ts(et, 128), :])
            pxv = tc.alloc_tile_pool(name="pxv", bufs=2)
            for sc in range(4):          # 512-wide column chunks
                xv_c = pxv.tile([128, ET, 512], f32r, tag="xv", name="xv_c")
                for et in range(ET):
                    nc.sync.dma_start(
                        xv_c[:, et, :], xv_d.ap()[ts(et, 128), ts(sc, 512)])
                for sti in range(4):
                    st = sc * 4 + sti
                    for ac in range(AC):
                        ps = pp512.tile([128, 512], f32, tag="ps", name="ps_b")
                        for et in range(ET):
                            nc.tensor.matmul(
                                ps[:], xv_c[:, et, ts(sti, 128)],
                                wv[:, et, ts(ac, 512)],
                                start=(et == 0), stop=(et == ET - 1),
                            )
                        nc.any.tensor_copy(v_sb[:, st, ts(ac, 512)], ps[:])
            pxv.release()

            # ---- Phase B': kT[a, k] = (key @ Wk + bk)^T -> DRAM ----
            pxk = tc.alloc_tile_pool(name="pxk", bufs=2)
            pkst = tc.alloc_tile_pool(name="pkst", bufs=2)
            for kc in range(KC):
                xk_t = pxk.tile([128, ET, 512], f32r, tag="xk", name="xk_t")
                for et in range(ET):
                    nc.sync.dma_start(
                        xk_t[:, et, :], xk_d.ap()[ts(et, 128), ts(kc, 512)])
                for at in range(AT):
                    ps = pp512.tile([128, 512], f32, tag="ps", name="ps_k")
                    for et in range(ET):
                        nc.tensor.matmul(
                            ps[:], wk[:, et, ts(at, 128)], xk_t[:, et, :],
                            start=(et == 0), stop=(et == ET - 1),
                        )
                    kst = pkst.tile([128, 512], f32r, tag="kst", name="kst")
                    nc.scalar.activation(
                        kst[:], ps[:], Act.Identity,
                        bias=bkt[:, at:at + 1], scale=1.0,
                    )
                    nc.scalar.dma_start(ktp[ts(at, 128), ts(kc, 512)], kst[:])
            pkst.release()
            pxk.release()
            pwk.release()
            pwv.release()

            # ---- Phase C: scores^T, exp, denominators, probs @ V ----
            with (
                tc.tile_pool(name="pe", bufs=1) as pe,
                tc.tile_pool(name="pktp", bufs=2) as pktp,
                tc.tile_pool(name="pcm", bufs=1) as pcm,
                tc.tile_pool(name="pot", bufs=2) as pot,
            ):
                bvb = pcm.tile([128, A], f32)
                nc.gpsimd.dma_start(bvb[:], bvb_d.ap()[:, :])
                acc = pcm.tile([128, SQ], f32r)

                E_t = pe.tile([128, KT, SQ], f32r)  # [k-part, kt, q]
                for kc in range(KC):
                    ktp_t = pktp.tile([128, AT, 512], f32r, tag="ktp",
                                      name="ktp_t")
                    for at in range(AT):
                        nc.sync.dma_start(
                            ktp_t[:, at, :], ktp[ts(at, 128), ts(kc, 512)])
                    for ki in range(4):
                        kt = kc * 4 + ki
                        psc = pps.tile([128, SQ], f32, tag="psc", name="psc")
                        for at in range(AT):
                            for qc in range(QC):
                                nc.tensor.matmul(
                                    psc[:, ts(qc, 512)],
                                    ktp_t[:, at, ts(ki, 128)],
                                    qT[:, at, ts(qc, 512)],
                                    start=(at == 0), stop=(at == AT - 1),
                                )
                        nc.scalar.activation(
                            E_t[:, kt, :], psc[:], Act.Exp,
                            bias=0.0, scale=SCALE)

                # denominators
                nc.vector.tensor_tensor(
                    acc[:], E_t[:, 0, :], E_t[:, 1, :], Alu.add)
                for kt in range(2, KT):
                    nc.vector.tensor_tensor(
                        acc[:], acc[:], E_t[:, kt, :], Alu.add)
                for qs in range(QS):
                    psd = ppd.tile([128, 2], f32, tag="psd", name="psd")
                    nc.tensor.matmul(
                        psd[:], acc[:, ts(qs, 128)], ones[:],
                        start=True, stop=True)
                    nc.vector.reciprocal(recip[:, qs:qs + 1], psd[:, 0:1])

                for ac in range(AC):
                    for qs in range(QS):
                        ps = pp512.tile([128, 512], f32, tag="ps", name="ps_av")
                        for kt in range(KT):
                            nc.tensor.matmul(
                                ps[:], E_t[:, kt, ts(qs, 128)],
                                v_sb[:, kt, ts(ac, 512)],
                                start=(kt == 0), stop=(kt == KT - 1),
                            )
                        ot = pot.tile([128, 512], f32, tag="ot", name="ot")
                        nc.vector.tensor_scalar(
                            ot[:], ps[:], recip[:, qs:qs + 1], None, Alu.mult)
                        nc.vector.tensor_tensor(
                            ot[:], ot[:], bvb[:, ts(ac, 512)], Alu.add)
                        nc.sync.dma_start(
                            out_d.ap()[ts(qs, 128), ts(ac, 512)], ot[:])
            pwv_dummy = None  # C pools closed by with-block
            pv.release()

    nc.compile()
    return nc


_nc_cache = None


def _get_nc():
    global _nc_cache
    if _nc_cache is None:
        _nc_cache = build()
    return _nc_cache


def kernel(query, key, value, Wq, bq, Wk, bk, Wv, bv):
    query = np.asarray(query, dtype=np.float32)
    key = np.asarray(key, dtype=np.float32)
    value = np.asarray(value, dtype=np.float32)
    Wq = np.ascontiguousarray(np.asarray(Wq, dtype=np.float32))
    Wk = np.ascontiguousarray(np.asarray(Wk, dtype=np.float32))
    Wv = np.ascontiguousarray(np.asarray(Wv, dtype=np.float32))
    bq = np.asarray(bq, dtype=np.float32)
    bk = np.asarray(bk, dtype=np.float32)
    bv = np.asarray(bv, dtype=np.float32)

    nc = _get_nc()

    bqt = np.ascontiguousarray(bq.reshape(AT, 128).T)
    bkt = np.ascontiguousarray(bk.reshape(AT, 128).T)
    bvb = np.ascontiguousarray(np.broadcast_to(bv, (128, A)))
    ones = np.ones((128, 2), np.float32)

    kTs = [np.ascontiguousarray(key[b].T) for b in range(B)]
    vTs = [np.ascontiguousarray(value[b].T) for b in range(B)]

    in_maps = []
    for c in range(8):
        b, h = c // 2, c % 2
        in_maps.append({
            "xq": np.ascontiguousarray(query[b, h * SQ:(h + 1) * SQ, :].T),
            "xk": kTs[b],
            "xv": vTs[b],
            "wq": Wq, "wk": Wk, "wv": Wv,
            "bqt": bqt, "bkt": bkt, "bvb": bvb, "ones": ones,
        })

    global _last_in_maps
    _last_in_maps = in_maps
    res = bass_utils.run_bass_kernel_spmd(nc, in_maps, core_ids=list(range(8)))

    out = np.empty((B, S, A), np.float32)
    for c in range(8):
        b, h = c // 2, c % 2
        out[b, h * SQ:(h + 1) * SQ, :] = res.results[c]["out"]
    return out
